# revision 51
# baseline (speedup 1.0000x reference)
"""MoE feed-forward (noisy top-2 gating over 64 experts) on 8 TRN2 NeuronCores.

Strategy (two device phases, host does only the 64-way top-2 bookkeeping):
  Phase 1 (device): tokens sharded 2048/core. Each core computes its shard's
    gate logits  x @ [gate_w | gate_noise_w]  in fp8 (f32 PSUM accumulate),
    applies softplus/noise, reduces over its tokens -> [64] partial sums,
    PE-transposes to [1,64] and stores with a single DMA descriptor.
  Host: sums the 8 partials -> mean logits, top-2 + softmax (matches
    jax.lax.top_k tie semantics via stable argsort), slices + bf16-casts the
    two selected experts' weight tables.
  Phase 2 (device): per core, hT = relu(g_e * (x @ Wi_e)) for both experts
    (gates folded into the relu scale), then out^T[do,tok] = sum_e Wo_e^T hT_e
    with Wo stationary in the PE (4 token-chunks streamed per weight load).
    Output is stored transposed in bf16; the host un-transposes for free.

Perf notes vs the previous version:
  - x is staged host-side in chunk-major contiguous layout so every chunk
    DMA is 128 x 4-8KB descriptors (line rate) instead of 512B scatter.
  - FFN1/FFN2 loops are ordered so consecutive matmuls share the stationary
    operand; a post-BIR pass drops the redundant LDWEIGHTS (each costs
    ~46ns of PE issue time; >700 of them in the old schedule).
  - Phase-1's [64,1] result is PE-transposed to [1,64] so the final store is
    one descriptor instead of 16 (the scattered store waited ~5us).

All matmuls run with fp32 PSUM accumulation (measured end-to-end rel err
~3e-3 vs the fp32 reference; top-2 selection margin is ~4000x the bf16 gate
error on the reference input distribution).
"""

import sys

for _p in ("/opt/trn_rl_repo", "/root/.axon_site/_ro/trn_rl_repo"):
    if _p not in sys.path:
        sys.path.insert(0, _p)

import ml_dtypes
import numpy as np

import concourse.bass as bass
import concourse.mybir as mybir
import concourse.tile as tile
from concourse import bass_utils
from concourse.bass_utils import run_bass_kernel_spmd


def _patch_walrus_args():
    """Allow injecting extra walrus_driver flags via EXTRA_WALRUS_ARGS
    (comma-separated).  Flags given here replace an existing flag with the
    same --name if present."""
    import os

    orig = bass_utils.bir_verify_and_optimise
    if getattr(bass_utils, "_walrus_patched", False):
        return

    def patched(tmpdir, inp="bir.json", outp="file.neff", arch=None, *,
                dve_root=None):
        extra = os.environ.get("EXTRA_WALRUS_ARGS", "")
        if not extra:
            return orig(tmpdir, inp=inp, outp=outp, arch=arch,
                        dve_root=dve_root)
        import concourse.bass_utils as bu

        run_command_orig = bu.run_command

        def run_command_patched(cmd, **kw):
            if cmd and str(cmd[0]).endswith("walrus_driver"):
                new = list(cmd)
                for flag in extra.split(","):
                    if not flag:
                        continue
                    name = flag.split("=")[0]
                    new = [a for a in new if not str(a).startswith(name)]
                    new.append(flag)
                cmd = new
            return run_command_orig(cmd, **kw)

        bu.run_command = run_command_patched
        try:
            return orig(tmpdir, inp=inp, outp=outp, arch=arch,
                        dve_root=dve_root)
        finally:
            bu.run_command = run_command_orig

    bass_utils.bir_verify_and_optimise = patched
    bass_utils._walrus_patched = True


_patch_walrus_args()


def _ensure_ntff_hook():
    """Make trace=True / BASS_TRACE profiling work even when the image's
    antenv package lacks axon_hooks (boot then skips hook registration)."""
    try:
        import antenv.axon_hooks  # noqa: F401
        return
    except ImportError:
        pass
    try:
        import types

        import antenv

        mod = types.ModuleType("antenv.axon_hooks")
        mod._hook = None

        def set_axon_ntff_profile_hook(hook):
            mod._hook = hook

        def get_axon_ntff_profile_hook():
            return mod._hook

        mod.set_axon_ntff_profile_hook = set_axon_ntff_profile_hook
        mod.get_axon_ntff_profile_hook = get_axon_ntff_profile_hook
        sys.modules["antenv.axon_hooks"] = mod
        antenv.axon_hooks = mod
        from trn_agent_boot.trn_boot import _ntff_profile_via_ctypes

        mod._hook = _ntff_profile_via_ctypes("/opt/axon/libaxon_pjrt.so")
    except Exception:
        pass  # profiling degrades gracefully; execution is unaffected


_ensure_ntff_hook()

# ---------------------------------------------------------------- shapes
B, L, D_IN, D_HID, D_OUT = 4, 4096, 1024, 1024, 1024
E, TOPK = 64, 2
N_CORES = 8
T = B * L            # 16384 tokens
TC = T // N_CORES    # 2048 tokens per core
CH = 512             # token chunk (matmul moving free dim = one PSUM bank)
NCH = TC // CH       # 4 chunks per core
KB = D_IN // 128     # 8 contraction blocks
HB = D_HID // 128    # 8 hidden blocks
NB = D_OUT // 128    # 8 output blocks

F32 = mybir.dt.float32
BF16 = mybir.dt.bfloat16
FP8 = mybir.dt.float8e4  # ml_dtypes.float8_e4m3

# ------------------------------------------------- walrus workaround
# The walrus build in this container supports only ONE sync-wait command
# per instruction; Tile attaches multi-wait lists.  Split them: the tail
# drain via a patched _drain_and_barrier, everything else via a BIR
# post-pass inserting single-wait NoOps ahead of multi-wait instructions.
_TILE_PATCHED = False


def _patch_tile_drain():
    global _TILE_PATCHED
    if _TILE_PATCHED:
        return
    _TILE_PATCHED = True

    def _drain_and_barrier(self, tick_clock, wait_clock):
        n1 = self.nc.sync.nop(nofuse=True)
        wait_clock.add_sem_waits(
            n1.ins, tile.ScopedClock({None: tick_clock.global_clock})
        )
        waits = list(n1.ins.sync_info.on_wait) if n1.ins.sync_info else []
        if len(waits) > 1:
            n1.ins.sync_info.on_wait = waits[:1]
            for i in range(1, len(waits)):
                nx = self.nc.sync.nop(nofuse=True)
                nx.ins.sync_info = mybir.SyncInfo(on_wait=[waits[i]], on_update=[])
        self.nc.sync.drain()
        self.nc.all_engine_barrier()
        assert self.sems is not None
        popped = self.nc._tile_sem_poison_stack.pop()
        assert popped is self._sem_poison
        # python-side bookkeeping only — the device-side clear
        # (gpsimd dma_reset + sem RANGE_CLEAR, ~2-3us of q7-launch
        # latency) and the trailing barrier are redundant with the
        # walrus postamble's full semaphore wipe
        sems = [s.num if hasattr(s, "num") else s
                for s in self.sems.allocated().values()]
        self.nc._state.prepend_free_semaphores(sems)
        for poison_set in self.nc._tile_sem_poison_stack:
            poison_set.update(sems)

    tile.TileContext._drain_and_barrier = _drain_and_barrier


def _split_multi_waits(nc):
    n_split = 0
    for f in nc.m.functions:
        for bb in f.blocks:
            insts = list(bb.instructions)
            out = []
            for inst in insts:
                si = inst.sync_info
                if si is not None and si.on_wait and len(si.on_wait) > 1:
                    waits = list(si.on_wait)
                    for w in waits[:-1]:
                        nop = mybir.InstNoOp(
                            name=f"{inst.name}-ws{n_split}", ins=[], outs=[]
                        )
                        nop.engine = inst.engine
                        nop.sync_info = mybir.SyncInfo(on_wait=[w], on_update=[])
                        out.append(nop)
                        n_split += 1
                    si.on_wait = waits[-1:]
                out.append(inst)
            if len(out) != len(insts):
                bb.instructions[:] = out
    return n_split


def _dedupe_ldweights(nc):
    """Drop InstLdweights that reload the exact weights already resident in
    the PE array (same AP/mode as the immediately preceding load, with no
    intervening PE-array-clobbering op).  The PE keeps its stationary
    operand across matmuls, so consecutive matmuls sharing lhsT only need
    the first load; each removed LDWEIGHTS saves ~46ns of PE issue time.
    Sync waits/updates on a removed load are transferred to the following
    instruction (the matmul), then _split_multi_waits handles overflow."""
    n_removed = 0
    for f in nc.m.functions:
        for bb in f.blocks:
            insts = list(bb.instructions)
            out = []
            prev_sig = None
            carry = None  # sync_info of removed LDW awaiting a new home
            for inst in insts:
                is_pe = getattr(inst, "engine", None) == mybir.EngineType.PE
                if isinstance(inst, mybir.InstLdweights):
                    sig = (
                        str(inst.ins[0]),
                        str(inst.perf_mode),
                        bool(inst.is_transpose),
                    )
                    if inst.is_transpose:
                        prev_sig = None
                        out.append(inst)
                        continue
                    if sig == prev_sig:
                        si = inst.sync_info
                        if si is not None and (si.on_wait or si.on_update):
                            if carry is None:
                                carry = mybir.SyncInfo(on_wait=[], on_update=[])
                            carry.on_wait.extend(si.on_wait)
                            carry.on_update.extend(si.on_update)
                        n_removed += 1
                        continue
                    prev_sig = sig
                    out.append(inst)
                    continue
                if is_pe:
                    if isinstance(inst, mybir.InstMatmult):
                        if inst.is_transpose:
                            prev_sig = None
                    elif not isinstance(
                        inst, (mybir.InstEventSemaphore, mybir.InstNoOp)
                    ):
                        # drains/branches/etc: don't assume array state
                        prev_sig = None
                    if carry is not None:
                        si = inst.sync_info
                        if si is None:
                            inst.sync_info = carry
                        else:
                            si.on_wait.extend(carry.on_wait)
                            si.on_update.extend(carry.on_update)
                        carry = None
                out.append(inst)
            assert carry is None, "removed LDW waits had no following PE inst"
            if len(out) != len(insts):
                bb.instructions[:] = out
    return n_removed


# ------------------------------------------------------------ builders
def _build_phase1():
    """Gate partials: per core [1,64] f32 = sum over its 2048 tokens of
    x@gate_w + softplus(x@gate_noise_w)*noise   (fp8 matmul, f32 psum).

    fp8-e4m3 is safe here: quantization noise averages over 16384 tokens
    (measured mean-logit err 1.3e-3 vs 0.216 top-2/3 margin, and 8e-5
    absolute error on the softmax gates)."""
    _patch_tile_drain()
    nc = bass.Bass("TRN2", target_bir_lowering=False, debug=False,
                   num_devices=N_CORES)
    # host layout: row p = [c][db][t] so each chunk DMA is one contiguous
    # 4KB segment per partition (line rate; the old (db p) t rearrange
    # produced 512B descriptors and ran at ~100GB/s)
    xt_in = nc.dram_tensor("xt", [128, NCH * KB * CH], FP8,
                           kind="ExternalInput")
    gw = nc.dram_tensor("gw", [128, KB * 128], FP8, kind="ExternalInput")
    noise = nc.dram_tensor("noise", [E, 1], F32, kind="ExternalInput")
    ident = nc.dram_tensor("ident", [E, E], F32, kind="ExternalInput")
    part = nc.dram_tensor("part", [1, E], F32, kind="ExternalOutput")

    with tile.TileContext(nc) as tc:
        with (
            tc.tile_pool(name="const", bufs=1) as const,
            tc.tile_pool(name="xt", bufs=1) as xtp,
            tc.tile_pool(name="ps", bufs=1, space="PSUM") as psp,
            tc.tile_pool(name="sb", bufs=3) as sbp,
            tc.tile_pool(name="red", bufs=NCH + 2) as redp,
        ):
            # gw staged as [p, db-pair, 2, 128] for DoubleRow matmuls
            # (fp8 high-perf mode: 2 contraction k-tiles per instruction)
            gw_sb = const.tile([128, KB // 2, 2, 128], FP8)
            nc.scalar.dma_start(out=gw_sb[:], in_=gw[:])
            noise_sb = const.tile([E, 1], F32)
            nc.scalar.dma_start(out=noise_sb[:], in_=noise[:])
            ident_sb = const.tile([E, E], F32)
            nc.scalar.dma_start(out=ident_sb[:], in_=ident[:])

            # load chunk PAIRS (8KB rows — 4KB fp8 rows halve the queue
            # service rate) on the two fast FIFOs, consts on scalar
            pair_tiles = []
            for p_ in range(2):
                pt_ = xtp.tile([128, 2 * KB, CH], FP8, tag=f"xp{p_}",
                               name=f"xp{p_}")
                (nc.sync if p_ == 0 else nc.gpsimd).dma_start(
                    out=pt_[:],
                    in_=xt_in[:, p_ * 2 * KB * CH:(p_ + 1) * 2 * KB * CH],
                )
                pair_tiles.append(pt_)

            def xt_rhs(c, db):
                return pair_tiles[c // 2][:, (c % 2) * KB + db, :]

            # PE warmup while DMAs stage (HAM clock gate -> 8/8), long
            # enough to bridge to the first chunk's arrival so the PE
            # doesn't idle (idling drops the clock back to mid-pstate);
            # offsets varied so the LDW-dedupe pass keeps each load.
            wz = const.tile([128, 512], BF16, tag="warm")
            nc.vector.memset(wz[:], 0.0)
            pw = psp.tile([128, 512], F32, space="PSUM", tag="warm")
            NW = 40
            for i in range(NW):
                o = (i % 4) * 128
                nc.tensor.matmul(pw[:, :128], lhsT=wz[:, o:o + 128],
                                 rhs=wz[:, :128],
                                 start=(i == 0), stop=(i == NW - 1))

            partials = []
            for c in range(NCH):
                ps_g = psp.tile([128, CH], F32, space="PSUM", tag="g",
                                name=f"ps_g{c}", bufs=3)
                for db2 in range(KB // 2):
                    base = (c % 2) * KB + 2 * db2
                    nc.tensor.matmul(
                        ps_g[:], lhsT=gw_sb[:, db2, :, :],
                        rhs=pair_tiles[c // 2][:, base:base + 2, :],
                        start=(db2 == 0), stop=(db2 == KB // 2 - 1),
                        perf_mode=mybir.MatmulPerfMode.DoubleRow,
                    )
                # softplus(v) = ln(exp(v) + 1) — this walrus's ACT tables
                # have no native softplus; exp/ln share one func set.
                # Gate pre-activations are O(10), so exp cannot overflow.
                ex = sbp.tile([E, CH], F32)
                nc.scalar.activation(
                    ex[:], ps_g[E:2 * E, :], mybir.ActivationFunctionType.Exp,
                )
                sp = sbp.tile([E, CH], F32)
                nc.scalar.activation(
                    sp[:], ex[:], mybir.ActivationFunctionType.Ln, bias=1.0,
                )
                comb = sbp.tile([E, CH], F32)
                pc = redp.tile([E, 1], F32, tag="partial")
                nc.vector.scalar_tensor_tensor(
                    out=comb[:], in0=sp[:], scalar=noise_sb[:, :1],
                    in1=ps_g[:E, :],
                    op0=mybir.AluOpType.mult, op1=mybir.AluOpType.add,
                    accum_out=pc[:],
                )
                partials.append(pc)
            while len(partials) > 1:
                nxt = []
                for i in range(0, len(partials) - 1, 2):
                    s = redp.tile([E, 1], F32, tag="sum")
                    nc.vector.tensor_add(s[:], partials[i][:], partials[i + 1][:])
                    nxt.append(s)
                if len(partials) % 2:
                    nxt.append(partials[-1])
                partials = nxt
            # [64,1] -> [1,64] on the PE so the store is one descriptor
            # (the scattered 64-partition store waited ~5us on completion)
            pt = psp.tile([1, E], F32, space="PSUM", tag="pt")
            nc.tensor.transpose(pt[:], partials[0][:], ident_sb[:])
            row = redp.tile([1, E], F32, tag="row")
            nc.vector.tensor_copy(row[:], pt[:])
            nc.sync.dma_start(out=part[:], in_=row[:])

    _dedupe_ldweights(nc)
    _split_multi_waits(nc)
    return nc


def _build_phase2(with_bo):
    """FFN over the two selected experts, token-sharded, gates folded in.

    FFN1: hT[e,h] = relu(g_e*(x @ Wi_e))^T per 128-row h-block, psum [dh,tok].
    FFN2 runs transposed: out^T[do,tok] += Wo[e,h,do]^T @ hT[e,h] with the
    Wo tile stationary, streaming all four 512-token chunks per load; the
    host un-transposes the bf16 result for free.

    Loop order maximizes stationary-operand reuse (LDW dedupe): FFN1 e0/c0
    runs db-outer so the PE consumes wi0 parts the moment they land, the
    rest runs h-outer with db inner and token-chunks innermost.
    """
    _patch_tile_drain()
    nc = bass.Bass("TRN2", target_bir_lowering=False, debug=False,
                   num_devices=N_CORES)
    # host layout: row p = [c][db][t], contiguous 8KB per partition chunk
    xt_in = nc.dram_tensor("xt", [128, NCH * KB * CH], BF16,
                           kind="ExternalInput")
    # host-contiguous layouts: row p holds every block's slice for that
    # partition, so each load is 128 long contiguous descriptors
    wi = nc.dram_tensor("wi", [TOPK, 128, KB * D_HID], BF16,
                        kind="ExternalInput")
    wo = nc.dram_tensor("wo", [TOPK, 128, HB * D_OUT], BF16,
                        kind="ExternalInput")
    scales = nc.dram_tensor("scales", [128, TOPK], F32, kind="ExternalInput")
    bias1 = nc.dram_tensor("bias1", [128, TOPK * HB], F32, kind="ExternalInput")
    if with_bo:
        bo_g = nc.dram_tensor("bo_g", [1, D_OUT], BF16, kind="ExternalInput")
    # transposed output, bf16; host transposes back (free) and upcasts
    out = nc.dram_tensor("out", [D_OUT, TC], BF16, kind="ExternalOutput")

    with tile.TileContext(nc) as tc:
        with (
            tc.tile_pool(name="const", bufs=1) as const,
            tc.tile_pool(name="xt", bufs=1) as xtp,
            tc.tile_pool(name="ps", bufs=1, space="PSUM") as ps,
            tc.tile_pool(name="ht", bufs=NCH) as htp,
            tc.tile_pool(name="ob", bufs=2) as obp,
        ):
            # Per-core DMA is bandwidth-capped and each ACTIVE queue gets
            # an ~equal share, so the startup-critical 6MB (x + wi0) is
            # spread evenly over all three queues in deadline order, and
            # the late-needed loads (wi1, wo) are queued BEHIND them on
            # the same queues.  Queue service rate also scales with
            # descriptor size — keep rows >= 4KB.
            #   sync:   xc0h0, xc1, xc3h0   then wo0
            #   gpsimd: wi0h0, wi0h1        then wi1
            #   scalar: consts, xc0h1, xc2, xc3h1   then wo1 (deferred)
            scales_sb = const.tile([128, TOPK], F32)
            nc.scalar.dma_start(out=scales_sb[:], in_=scales[:])
            bias1_sb = const.tile([128, TOPK * HB], F32)
            nc.scalar.dma_start(out=bias1_sb[:], in_=bias1[:])
            if with_bo:
                bo_sb = const.tile([1, D_OUT], BF16)
                nc.scalar.dma_start(out=bo_sb[:], in_=bo_g[:])
                ones_sb = const.tile([1, CH], BF16)
                nc.vector.memset(ones_sb[:], 1.0)
            # Startup DMA layout (empirically best):
            #   gpsimd: wi0h0, wi0h1, wi1
            #   sync:   xc0h0, xc1, xc3h0, wo0 (then the out stores)
            #   scalar: consts, xc0h1, xc2, xc3h1, (wo1 deferred)
            def _xdma(eng, tile_, lo, hi):
                eng.dma_start(out=tile_[:], in_=xt_in[:, lo * CH:hi * CH])

            wi0_halves = []
            for half in range(2):
                wq = const.tile([128, 4 * D_HID], BF16, tag=f"wi0h{half}",
                                name=f"wi0h{half}")
                nc.gpsimd.dma_start(
                    out=wq[:],
                    in_=wi[0, :, half * 4 * D_HID:(half + 1) * 4 * D_HID],
                )
                wi0_halves.append(wq)
            xc0_halves = []
            xc3_halves = []
            for half in range(2):
                xh = xtp.tile([128, KB // 2, CH], BF16, tag=f"xc0h{half}",
                              name=f"xc0h{half}")
                _xdma(nc.sync if half == 0 else nc.scalar,
                      xh, half * 4, (half + 1) * 4)
                xc0_halves.append(xh)
            xt_chunks = [None]
            for c in (1, 2):
                xc = xtp.tile([128, KB, CH], BF16, tag=f"xc{c}", name=f"xc{c}")
                _xdma(nc.sync if c == 1 else nc.scalar,
                      xc, c * KB, (c + 1) * KB)
                xt_chunks.append(xc)
            for half in range(2):
                xh = xtp.tile([128, KB // 2, CH], BF16, tag=f"xc3h{half}",
                              name=f"xc3h{half}")
                _xdma(nc.sync if half == 0 else nc.scalar,
                      xh, 3 * KB + half * 4, 3 * KB + (half + 1) * 4)
                xc3_halves.append(xh)
            wi1_sb = const.tile([128, KB * D_HID], BF16)
            nc.gpsimd.dma_start(out=wi1_sb[:], in_=wi[1])
            wo0_sb = const.tile([128, HB * D_OUT], BF16)
            nc.sync.dma_start(out=wo0_sb[:], in_=wo[0])
            # wo1 allocated now, its load ISSUED from the scalar engine
            # after the pass-A relus (so its issue can't block the scalar
            # sequencer while x loads are still queued)
            wo1_sb = const.tile([128, HB * D_OUT], BF16)
            wo_sb = [wo0_sb, wo1_sb]

            # PE warmup while DMAs stage (HAM -> 8/8 before real matmuls);
            # offsets varied so LDW dedupe keeps each load.
            wz = const.tile([128, 512], BF16, tag="warm")
            nc.vector.memset(wz[:], 0.0)
            pw = ps.tile([128, 512], F32, space="PSUM", tag="ps", bufs=8,
                         name="warm")
            NW = 44
            for i in range(NW):
                o = (i % 4) * 128
                nc.tensor.matmul(pw[:, :128], lhsT=wz[:, o:o + 128],
                                 rhs=wz[:, :128],
                                 start=(i == 0), stop=(i == NW - 1))

            def wi_lhsT(e, db, h):
                if e == 0:
                    return wi0_halves[db // 4][
                        :, (db % 4) * D_HID + h * 128:
                        (db % 4) * D_HID + (h + 1) * 128]
                return wi1_sb[:, db * D_HID + h * 128:
                              db * D_HID + (h + 1) * 128]

            def xt_rhs(db, c):
                if c == 0:
                    return xc0_halves[db // 4][:, db % 4, :]
                if c == 3:
                    return xc3_halves[db // 4][:, db % 4, :]
                return xt_chunks[c][:, db, :]

            ht_tiles = {}

            def ht_of(c):
                if c not in ht_tiles:
                    ht_tiles[c] = htp.tile([128, TOPK * HB, CH], BF16,
                                           tag="ht", name=f"ht{c}")
                return ht_tiles[c]

            def relu_out(c, e, h, ph):
                nc.scalar.activation(
                    ht_of(c)[:, e * HB + h, :], ph[:],
                    mybir.ActivationFunctionType.Relu,
                    bias=bias1_sb[:, e * HB + h:e * HB + h + 1],
                    scale=scales_sb[:, e:e + 1],
                )

            # --- FFN1 pass A: (e0, c0) with all 8 h-tiles' accumulation
            # groups interleaved: both h-groups consume db 0-3 (first wi0
            # half + first xc0 half) before either touches db 4-7, pushing
            # the second halves' DMA deadline ~4us later.  The kernel
            # start is DMA-paced, so the PE chases arrivals here.
            phsA = [
                ps.tile([128, CH], F32, space="PSUM", tag="ps",
                        name=f"phA_{j}", bufs=8)
                for j in range(4)
            ]
            phsB = [
                ps.tile([128, CH], F32, space="PSUM", tag="ps",
                        name=f"phB_{j}", bufs=8)
                for j in range(4)
            ]
            for half, phs, hbase in ((0, phsA, 0), (0, phsB, 4),
                                     (1, phsA, 0), (1, phsB, 4)):
                for db in range(half * 4, half * 4 + 4):
                    for j in range(4):
                        nc.tensor.matmul(
                            phs[j][:], lhsT=wi_lhsT(0, db, hbase + j),
                            rhs=xt_rhs(db, 0),
                            start=(db == 0), stop=(db == KB - 1),
                        )
                if half == 1:
                    for j in range(4):
                        relu_out(0, 0, hbase + j, phs[j])
            nc.scalar.dma_start(out=wo1_sb[:], in_=wo[1])

            # --- FFN1 main: h-outer, db inner, chunk-group innermost so
            # each wi tile loads once and streams the whole group (LDW
            # dedupe).  All psum tiles share one 8-deep bank rotation, so
            # the groups stay double-buffered against the relu drain.
            def ffn1_h(e, h, chunks):
                phs = {
                    c: ps.tile([128, CH], F32, space="PSUM", tag="ps",
                               name=f"ph{e}_{h}_{c}", bufs=8)
                    for c in chunks
                }
                for db in range(KB):
                    for c in chunks:
                        nc.tensor.matmul(
                            phs[c][:], lhsT=wi_lhsT(e, db, h),
                            rhs=xt_rhs(db, c),
                            start=(db == 0), stop=(db == KB - 1),
                        )
                for c in chunks:
                    relu_out(c, e, h, phs[c])

            for h in range(HB):
                ffn1_h(0, h, range(1, NCH))
            for h in range(HB):
                ffn1_h(1, h, range(NCH))

            # --- FFN2 transposed: out^T[do,tok] = sum_{e,h} Wo^T @ hT
            # (+ bo_g ⊗ ones).  Wo tile stationary, all four chunks
            # streamed per load; psum [do=128, tok=512] per (do, chunk).
            n_mm = TOPK * HB
            for n in range(NB):
                ob = obp.tile([128, TC], BF16, tag="ob", name=f"ob{n}")
                pos = {
                    c: ps.tile([128, CH], F32, space="PSUM", tag="ps",
                               name=f"po{n}_{c}", bufs=8)
                    for c in range(NCH)
                }
                k = 0
                for e in range(TOPK):
                    for h in range(HB):
                        k += 1
                        for c in range(NCH):
                            nc.tensor.matmul(
                                pos[c][:],
                                lhsT=wo_sb[e][:, h * D_OUT + n * 128:
                                              h * D_OUT + (n + 1) * 128],
                                rhs=ht_tiles[c][:, e * HB + h, :],
                                start=(k == 1),
                                stop=(not with_bo and k == n_mm),
                            )
                if with_bo:
                    for c in range(NCH):
                        nc.tensor.matmul(
                            pos[c][:],
                            lhsT=bo_sb[:, n * 128:(n + 1) * 128],
                            rhs=ones_sb[:], start=False, stop=True,
                        )
                # psum->sbuf bf16 copies split across the (otherwise idle)
                # vector and scalar engines so they drain in ~half the time
                for c in range(NCH):
                    dst = ob[:, c * CH:(c + 1) * CH]
                    if c % 2 == 0:
                        nc.vector.tensor_copy(dst, pos[c][:])
                    else:
                        nc.scalar.activation(
                            dst, pos[c][:],
                            mybir.ActivationFunctionType.Copy,
                        )
                if n < NB - 1:
                    for lo in (0, 2 * CH):
                        nc.sync.dma_start(
                            out=out[n * 128:(n + 1) * 128, lo:lo + 2 * CH],
                            in_=ob[:, lo:lo + 2 * CH],
                        )
                else:
                    # last block: store per chunk on alternating queues so
                    # the final stores (on the critical tail) issue in
                    # parallel and each covers only 128KB
                    for c in range(NCH):
                        eng = nc.sync if c % 2 == 0 else nc.gpsimd
                        eng.dma_start(
                            out=out[n * 128:(n + 1) * 128,
                                    c * CH:(c + 1) * CH],
                            in_=ob[:, c * CH:(c + 1) * CH],
                        )

    _dedupe_ldweights(nc)
    _split_multi_waits(nc)
    return nc


_CACHE = {}


def _phase(name, *args):
    key = (name, *args)
    if key not in _CACHE:
        _CACHE[key] = _build_phase1() if name == "p1" else _build_phase2(*args)
    return _CACHE[key]


def _bf16(a):
    return np.asarray(a, np.float32).astype(ml_dtypes.bfloat16)


def _chunk_major(shard):
    """[TC, D_IN] -> [128, NCH*KB*CH] with row p = [c][db][t], so every
    chunk DMA is a single contiguous segment per partition."""
    return np.ascontiguousarray(
        shard.reshape(NCH, CH, KB, 128).transpose(3, 0, 2, 1)
        .reshape(128, NCH * KB * CH)
    )


def kernel(x, noise, gate_w, gate_noise_w, Wi, bi, Wo, bo, _timing=None):
    x = np.asarray(x, np.float32)
    noise = np.asarray(noise, np.float32)
    gate_w = np.asarray(gate_w, np.float32)
    gate_noise_w = np.asarray(gate_noise_w, np.float32)
    bi = np.asarray(bi, np.float32)
    bo = np.asarray(bo, np.float32)

    xb = _bf16(x.reshape(T, D_IN))
    xt_shards = [_chunk_major(xb[c * TC:(c + 1) * TC]) for c in range(N_CORES)]
    core_ids = list(range(N_CORES))

    # ---- phase 1: gate partials (fp8 halves the gate-phase DMA)
    xf8 = x.reshape(T, D_IN).astype(ml_dtypes.float8_e4m3)
    xt8_shards = [
        _chunk_major(xf8[c * TC:(c + 1) * TC]) for c in range(N_CORES)
    ]
    gw_cat = np.concatenate([gate_w, gate_noise_w], axis=1).astype(
        ml_dtypes.float8_e4m3
    )
    # [p, db-pair, 2, 128] layout for DoubleRow (one contiguous row per
    # partition)
    gw_host = np.ascontiguousarray(
        gw_cat.reshape(KB // 2, 2, 128, 128).transpose(2, 0, 1, 3)
        .reshape(128, KB * 128)
    )
    noise_col = noise.reshape(E, 1)
    ident = np.eye(E, dtype=np.float32)
    in1 = [
        {"xt": xt8_shards[c], "gw": gw_host, "noise": noise_col,
         "ident": ident}
        for c in range(N_CORES)
    ]
    r1 = run_bass_kernel_spmd(_phase("p1"), in1, core_ids,
                              **(_timing or {}).get("p1", {}))
    mean_logits = (
        sum(r1.results[c]["part"][0, :].astype(np.float64)
            for c in range(N_CORES)) / T
    ).astype(np.float32)

    # ---- host routing: top-2 + softmax (stable => jax.lax.top_k ties)
    idx = np.argsort(-mean_logits, kind="stable")[:TOPK]
    tv = mean_logits[idx]
    ex = np.exp(tv - tv.max())
    gates = (ex / ex.sum()).astype(np.float32)

    # ---- phase 2: FFN on the two selected experts
    # [e, p, db*D + col] layout: one contiguous row per partition
    wi_sel = np.ascontiguousarray(
        _bf16(np.asarray(Wi)[idx]).reshape(TOPK, KB, 128, D_HID)
        .transpose(0, 2, 1, 3).reshape(TOPK, 128, KB * D_HID)
    )
    wo_sel = np.ascontiguousarray(
        _bf16(np.asarray(Wo)[idx]).reshape(TOPK, HB, 128, D_OUT)
        .transpose(0, 2, 1, 3).reshape(TOPK, 128, HB * D_OUT)
    )
    scales = np.broadcast_to(gates, (128, TOPK)).copy()
    # bias1[p, e*HB+h] = g_e * bi[e_sel, h*128+p]
    bias1 = (gates[:, None] * bi[idx]).reshape(TOPK, HB, 128)
    bias1 = np.ascontiguousarray(bias1.transpose(2, 0, 1).reshape(128, TOPK * HB))
    with_bo = bool(np.any(bo[idx]))
    in2 = [
        {
            "xt": xt_shards[c], "wi": wi_sel, "wo": wo_sel,
            "scales": scales, "bias1": bias1,
        }
        for c in range(N_CORES)
    ]
    if with_bo:
        bo_g = _bf16((gates[:, None] * bo[idx]).sum(0).reshape(1, D_OUT))
        for m in in2:
            m["bo_g"] = bo_g
    r2 = run_bass_kernel_spmd(_phase("p2", with_bo), in2, core_ids,
                              **(_timing or {}).get("p2", {}))
    out = np.concatenate(
        [np.asarray(r2.results[c]["out"]).astype(np.float32).T
         for c in range(N_CORES)], axis=0
    )

    if isinstance(_timing, dict):
        _timing["exec_ns"] = [r1.exec_time_ns, r2.exec_time_ns]
    return out.reshape(B, L, D_OUT).astype(np.float32, copy=False)


# revision 53
# speedup vs baseline: 1.0119x; 1.0119x over previous
"""MoE feed-forward (noisy top-2 gating over 64 experts) on 8 TRN2 NeuronCores.

Strategy (two device phases, host does only the 64-way top-2 bookkeeping):
  Phase 1 (device): tokens sharded 2048/core. Each core computes its shard's
    gate logits  x @ [gate_w | gate_noise_w]  in fp8 (f32 PSUM accumulate),
    applies softplus/noise, reduces over its tokens -> [64] partial sums,
    PE-transposes to [1,64] and stores with a single DMA descriptor.
  Host: sums the 8 partials -> mean logits, top-2 + softmax (matches
    jax.lax.top_k tie semantics via stable argsort), slices + bf16-casts the
    two selected experts' weight tables.
  Phase 2 (device): per core, hT = relu(g_e * (x @ Wi_e)) for both experts
    (gates folded into the relu scale), then out^T[do,tok] = sum_e Wo_e^T hT_e
    with Wo stationary in the PE (4 token-chunks streamed per weight load).
    Output is stored transposed in bf16; the host un-transposes for free.

Perf notes vs the previous version:
  - x is staged host-side in chunk-major contiguous layout so every chunk
    DMA is 128 x 4-8KB descriptors (line rate) instead of 512B scatter.
  - FFN1/FFN2 loops are ordered so consecutive matmuls share the stationary
    operand; a post-BIR pass drops the redundant LDWEIGHTS (each costs
    ~46ns of PE issue time; >700 of them in the old schedule).
  - Phase-1's [64,1] result is PE-transposed to [1,64] so the final store is
    one descriptor instead of 16 (the scattered store waited ~5us).

All matmuls run with fp32 PSUM accumulation (measured end-to-end rel err
~3e-3 vs the fp32 reference; top-2 selection margin is ~4000x the bf16 gate
error on the reference input distribution).
"""

import sys

for _p in ("/opt/trn_rl_repo", "/root/.axon_site/_ro/trn_rl_repo"):
    if _p not in sys.path:
        sys.path.insert(0, _p)

import ml_dtypes
import numpy as np

import concourse.bass as bass
import concourse.mybir as mybir
import concourse.tile as tile
from concourse import bass_utils
from concourse.bass_utils import run_bass_kernel_spmd


def _patch_walrus_args():
    """Allow injecting extra walrus_driver flags via EXTRA_WALRUS_ARGS
    (comma-separated).  Flags given here replace an existing flag with the
    same --name if present."""
    import os

    orig = bass_utils.bir_verify_and_optimise
    if getattr(bass_utils, "_walrus_patched", False):
        return

    def patched(tmpdir, inp="bir.json", outp="file.neff", arch=None, *,
                dve_root=None):
        extra = os.environ.get("EXTRA_WALRUS_ARGS", "")
        if not extra:
            return orig(tmpdir, inp=inp, outp=outp, arch=arch,
                        dve_root=dve_root)
        import concourse.bass_utils as bu

        run_command_orig = bu.run_command

        def run_command_patched(cmd, **kw):
            if cmd and str(cmd[0]).endswith("walrus_driver"):
                new = list(cmd)
                for flag in extra.split(","):
                    if not flag:
                        continue
                    name = flag.split("=")[0]
                    new = [a for a in new if not str(a).startswith(name)]
                    new.append(flag)
                cmd = new
            return run_command_orig(cmd, **kw)

        bu.run_command = run_command_patched
        try:
            return orig(tmpdir, inp=inp, outp=outp, arch=arch,
                        dve_root=dve_root)
        finally:
            bu.run_command = run_command_orig

    bass_utils.bir_verify_and_optimise = patched
    bass_utils._walrus_patched = True


_patch_walrus_args()


def _ensure_ntff_hook():
    """Make trace=True / BASS_TRACE profiling work even when the image's
    antenv package lacks axon_hooks (boot then skips hook registration)."""
    try:
        import antenv.axon_hooks  # noqa: F401
        return
    except ImportError:
        pass
    try:
        import types

        import antenv

        mod = types.ModuleType("antenv.axon_hooks")
        mod._hook = None

        def set_axon_ntff_profile_hook(hook):
            mod._hook = hook

        def get_axon_ntff_profile_hook():
            return mod._hook

        mod.set_axon_ntff_profile_hook = set_axon_ntff_profile_hook
        mod.get_axon_ntff_profile_hook = get_axon_ntff_profile_hook
        sys.modules["antenv.axon_hooks"] = mod
        antenv.axon_hooks = mod
        from trn_agent_boot.trn_boot import _ntff_profile_via_ctypes

        mod._hook = _ntff_profile_via_ctypes("/opt/axon/libaxon_pjrt.so")
    except Exception:
        pass  # profiling degrades gracefully; execution is unaffected


_ensure_ntff_hook()

# ---------------------------------------------------------------- shapes
B, L, D_IN, D_HID, D_OUT = 4, 4096, 1024, 1024, 1024
E, TOPK = 64, 2
N_CORES = 8
T = B * L            # 16384 tokens
TC = T // N_CORES    # 2048 tokens per core
CH = 512             # token chunk (matmul moving free dim = one PSUM bank)
NCH = TC // CH       # 4 chunks per core
KB = D_IN // 128     # 8 contraction blocks
HB = D_HID // 128    # 8 hidden blocks
NB = D_OUT // 128    # 8 output blocks

F32 = mybir.dt.float32
BF16 = mybir.dt.bfloat16
FP8 = mybir.dt.float8e4  # ml_dtypes.float8_e4m3

# ------------------------------------------------- walrus workaround
# The walrus build in this container supports only ONE sync-wait command
# per instruction; Tile attaches multi-wait lists.  Split them: the tail
# drain via a patched _drain_and_barrier, everything else via a BIR
# post-pass inserting single-wait NoOps ahead of multi-wait instructions.
_TILE_PATCHED = False


def _patch_tile_drain():
    global _TILE_PATCHED
    if _TILE_PATCHED:
        return
    _TILE_PATCHED = True

    def _drain_and_barrier(self, tick_clock, wait_clock):
        n1 = self.nc.sync.nop(nofuse=True)
        wait_clock.add_sem_waits(
            n1.ins, tile.ScopedClock({None: tick_clock.global_clock})
        )
        waits = list(n1.ins.sync_info.on_wait) if n1.ins.sync_info else []
        if len(waits) > 1:
            n1.ins.sync_info.on_wait = waits[:1]
            for i in range(1, len(waits)):
                nx = self.nc.sync.nop(nofuse=True)
                nx.ins.sync_info = mybir.SyncInfo(on_wait=[waits[i]], on_update=[])
        self.nc.sync.drain()
        self.nc.all_engine_barrier()
        assert self.sems is not None
        popped = self.nc._tile_sem_poison_stack.pop()
        assert popped is self._sem_poison
        # python-side bookkeeping only — the device-side clear
        # (gpsimd dma_reset + sem RANGE_CLEAR, ~2-3us of q7-launch
        # latency) and the trailing barrier are redundant with the
        # walrus postamble's full semaphore wipe
        sems = [s.num if hasattr(s, "num") else s
                for s in self.sems.allocated().values()]
        self.nc._state.prepend_free_semaphores(sems)
        for poison_set in self.nc._tile_sem_poison_stack:
            poison_set.update(sems)

    tile.TileContext._drain_and_barrier = _drain_and_barrier


def _split_multi_waits(nc):
    n_split = 0
    for f in nc.m.functions:
        for bb in f.blocks:
            insts = list(bb.instructions)
            out = []
            for inst in insts:
                si = inst.sync_info
                if si is not None and si.on_wait and len(si.on_wait) > 1:
                    waits = list(si.on_wait)
                    for w in waits[:-1]:
                        nop = mybir.InstNoOp(
                            name=f"{inst.name}-ws{n_split}", ins=[], outs=[]
                        )
                        nop.engine = inst.engine
                        nop.sync_info = mybir.SyncInfo(on_wait=[w], on_update=[])
                        out.append(nop)
                        n_split += 1
                    si.on_wait = waits[-1:]
                out.append(inst)
            if len(out) != len(insts):
                bb.instructions[:] = out
    return n_split


def _dedupe_ldweights(nc):
    """Drop InstLdweights that reload the exact weights already resident in
    the PE array (same AP/mode as the immediately preceding load, with no
    intervening PE-array-clobbering op).  The PE keeps its stationary
    operand across matmuls, so consecutive matmuls sharing lhsT only need
    the first load; each removed LDWEIGHTS saves ~46ns of PE issue time.
    Sync waits/updates on a removed load are transferred to the following
    instruction (the matmul), then _split_multi_waits handles overflow."""
    n_removed = 0
    for f in nc.m.functions:
        for bb in f.blocks:
            insts = list(bb.instructions)
            out = []
            prev_sig = None
            carry = None  # sync_info of removed LDW awaiting a new home
            for inst in insts:
                is_pe = getattr(inst, "engine", None) == mybir.EngineType.PE
                if isinstance(inst, mybir.InstLdweights):
                    sig = (
                        str(inst.ins[0]),
                        str(inst.perf_mode),
                        bool(inst.is_transpose),
                    )
                    if inst.is_transpose:
                        prev_sig = None
                        out.append(inst)
                        continue
                    if sig == prev_sig:
                        si = inst.sync_info
                        if si is not None and (si.on_wait or si.on_update):
                            if carry is None:
                                carry = mybir.SyncInfo(on_wait=[], on_update=[])
                            carry.on_wait.extend(si.on_wait)
                            carry.on_update.extend(si.on_update)
                        n_removed += 1
                        continue
                    prev_sig = sig
                    out.append(inst)
                    continue
                if is_pe:
                    if isinstance(inst, mybir.InstMatmult):
                        if inst.is_transpose:
                            prev_sig = None
                    elif not isinstance(
                        inst, (mybir.InstEventSemaphore, mybir.InstNoOp)
                    ):
                        # drains/branches/etc: don't assume array state
                        prev_sig = None
                    if carry is not None:
                        si = inst.sync_info
                        if si is None:
                            inst.sync_info = carry
                        else:
                            si.on_wait.extend(carry.on_wait)
                            si.on_update.extend(carry.on_update)
                        carry = None
                out.append(inst)
            assert carry is None, "removed LDW waits had no following PE inst"
            if len(out) != len(insts):
                bb.instructions[:] = out
    return n_removed


# ------------------------------------------------------------ builders
def _build_phase1():
    """Gate partials: per core [1,64] f32 = sum over its 2048 tokens of
    x@gate_w + softplus(x@gate_noise_w)*noise   (fp8 matmul, f32 psum).

    fp8-e4m3 is safe here: quantization noise averages over 16384 tokens
    (measured mean-logit err 1.3e-3 vs 0.216 top-2/3 margin, and 8e-5
    absolute error on the softmax gates)."""
    _patch_tile_drain()
    nc = bass.Bass("TRN2", target_bir_lowering=False, debug=False,
                   num_devices=N_CORES)
    # host layout: row p = [c][db][t] so each chunk DMA is one contiguous
    # 4KB segment per partition (line rate; the old (db p) t rearrange
    # produced 512B descriptors and ran at ~100GB/s)
    xt_in = nc.dram_tensor("xt", [128, NCH * KB * CH], FP8,
                           kind="ExternalInput")
    gw = nc.dram_tensor("gw", [128, KB * 128], FP8, kind="ExternalInput")
    noise = nc.dram_tensor("noise", [E, 1], F32, kind="ExternalInput")
    ident = nc.dram_tensor("ident", [E, E], F32, kind="ExternalInput")
    part = nc.dram_tensor("part", [1, E], F32, kind="ExternalOutput")

    with tile.TileContext(nc) as tc:
        with (
            tc.tile_pool(name="const", bufs=1) as const,
            tc.tile_pool(name="xt", bufs=1) as xtp,
            tc.tile_pool(name="ps", bufs=1, space="PSUM") as psp,
            tc.tile_pool(name="sb", bufs=3) as sbp,
            tc.tile_pool(name="red", bufs=NCH + 2) as redp,
        ):
            # gw staged as [p, db-pair, 2, 128] for DoubleRow matmuls
            # (fp8 high-perf mode: 2 contraction k-tiles per instruction)
            gw_sb = const.tile([128, KB // 2, 2, 128], FP8)
            nc.scalar.dma_start(out=gw_sb[:], in_=gw[:])
            noise_sb = const.tile([E, 1], F32)
            nc.scalar.dma_start(out=noise_sb[:], in_=noise[:])
            ident_sb = const.tile([E, E], F32)
            nc.scalar.dma_start(out=ident_sb[:], in_=ident[:])

            # load chunk PAIRS (8KB rows — 4KB fp8 rows halve the queue
            # service rate) on the two fast FIFOs, consts on scalar
            pair_tiles = []
            for p_ in range(2):
                pt_ = xtp.tile([128, 2 * KB, CH], FP8, tag=f"xp{p_}",
                               name=f"xp{p_}")
                (nc.sync if p_ == 0 else nc.gpsimd).dma_start(
                    out=pt_[:],
                    in_=xt_in[:, p_ * 2 * KB * CH:(p_ + 1) * 2 * KB * CH],
                )
                pair_tiles.append(pt_)

            def xt_rhs(c, db):
                return pair_tiles[c // 2][:, (c % 2) * KB + db, :]

            # PE warmup while DMAs stage (HAM clock gate -> 8/8), long
            # enough to bridge to the first chunk's arrival so the PE
            # doesn't idle (idling drops the clock back to mid-pstate);
            # offsets varied so the LDW-dedupe pass keeps each load.
            wz = const.tile([128, 512], BF16, tag="warm")
            nc.vector.memset(wz[:], 0.0)
            pw = psp.tile([128, 512], F32, space="PSUM", tag="warm")
            NW = 40
            for i in range(NW):
                o = (i % 4) * 128
                nc.tensor.matmul(pw[:, :128], lhsT=wz[:, o:o + 128],
                                 rhs=wz[:, :128],
                                 start=(i == 0), stop=(i == NW - 1))

            partials = []
            for c in range(NCH):
                ps_g = psp.tile([128, CH], F32, space="PSUM", tag="g",
                                name=f"ps_g{c}", bufs=3)
                for db2 in range(KB // 2):
                    base = (c % 2) * KB + 2 * db2
                    nc.tensor.matmul(
                        ps_g[:], lhsT=gw_sb[:, db2, :, :],
                        rhs=pair_tiles[c // 2][:, base:base + 2, :],
                        start=(db2 == 0), stop=(db2 == KB // 2 - 1),
                        perf_mode=mybir.MatmulPerfMode.DoubleRow,
                    )
                # softplus(v) = ln(exp(v) + 1) — this walrus's ACT tables
                # have no native softplus; exp/ln share one func set.
                # Gate pre-activations are O(10), so exp cannot overflow.
                ex = sbp.tile([E, CH], F32)
                nc.scalar.activation(
                    ex[:], ps_g[E:2 * E, :], mybir.ActivationFunctionType.Exp,
                )
                sp = sbp.tile([E, CH], F32)
                nc.scalar.activation(
                    sp[:], ex[:], mybir.ActivationFunctionType.Ln, bias=1.0,
                )
                comb = sbp.tile([E, CH], F32)
                pc = redp.tile([E, 1], F32, tag="partial")
                nc.vector.scalar_tensor_tensor(
                    out=comb[:], in0=sp[:], scalar=noise_sb[:, :1],
                    in1=ps_g[:E, :],
                    op0=mybir.AluOpType.mult, op1=mybir.AluOpType.add,
                    accum_out=pc[:],
                )
                partials.append(pc)
            while len(partials) > 1:
                nxt = []
                for i in range(0, len(partials) - 1, 2):
                    s = redp.tile([E, 1], F32, tag="sum")
                    nc.vector.tensor_add(s[:], partials[i][:], partials[i + 1][:])
                    nxt.append(s)
                if len(partials) % 2:
                    nxt.append(partials[-1])
                partials = nxt
            # [64,1] -> [1,64] on the PE so the store is one descriptor
            # (the scattered 64-partition store waited ~5us on completion)
            pt = psp.tile([1, E], F32, space="PSUM", tag="pt")
            nc.tensor.transpose(pt[:], partials[0][:], ident_sb[:])
            row = redp.tile([1, E], F32, tag="row")
            nc.vector.tensor_copy(row[:], pt[:])
            nc.sync.dma_start(out=part[:], in_=row[:])

    _dedupe_ldweights(nc)
    _split_multi_waits(nc)
    return nc


def _build_phase2(with_bo):
    """FFN over the two selected experts, token-sharded, gates folded in.

    FFN1: hT[e,h] = relu(g_e*(x @ Wi_e))^T per 128-row h-block, psum [dh,tok].
    FFN2 runs transposed: out^T[do,tok] += Wo[e,h,do]^T @ hT[e,h] with the
    Wo tile stationary, streaming all four 512-token chunks per load; the
    host un-transposes the bf16 result for free.

    Loop order maximizes stationary-operand reuse (LDW dedupe): FFN1 e0/c0
    runs db-outer so the PE consumes wi0 parts the moment they land, the
    rest runs h-outer with db inner and token-chunks innermost.
    """
    _patch_tile_drain()
    nc = bass.Bass("TRN2", target_bir_lowering=False, debug=False,
                   num_devices=N_CORES)
    # host layout: row p = [c][db][t], contiguous 8KB per partition chunk
    xt_in = nc.dram_tensor("xt", [128, NCH * KB * CH], BF16,
                           kind="ExternalInput")
    # host-contiguous layouts: row p holds every block's slice for that
    # partition, so each load is 128 long contiguous descriptors
    wi = nc.dram_tensor("wi", [TOPK, 128, KB * D_HID], BF16,
                        kind="ExternalInput")
    wo = nc.dram_tensor("wo", [TOPK, 128, HB * D_OUT], BF16,
                        kind="ExternalInput")
    scales = nc.dram_tensor("scales", [128, TOPK], F32, kind="ExternalInput")
    bias1 = nc.dram_tensor("bias1", [128, TOPK * HB], F32, kind="ExternalInput")
    if with_bo:
        bo_g = nc.dram_tensor("bo_g", [1, D_OUT], BF16, kind="ExternalInput")
    # transposed output, bf16; host transposes back (free) and upcasts
    out = nc.dram_tensor("out", [D_OUT, TC], BF16, kind="ExternalOutput")

    with tile.TileContext(nc) as tc:
        with (
            tc.tile_pool(name="const", bufs=1) as const,
            tc.tile_pool(name="xt", bufs=1) as xtp,
            tc.tile_pool(name="ps", bufs=1, space="PSUM") as ps,
            tc.tile_pool(name="ht", bufs=NCH) as htp,
            tc.tile_pool(name="ob", bufs=2) as obp,
        ):
            # Per-core DMA is bandwidth-capped and each ACTIVE queue gets
            # an ~equal share, so the startup-critical 6MB (x + wi0) is
            # spread evenly over all three queues in deadline order, and
            # the late-needed loads (wi1, wo) are queued BEHIND them on
            # the same queues.  Queue service rate also scales with
            # descriptor size — keep rows >= 4KB.
            #   sync:   xc0h0, xc1, xc3h0   then wo0
            #   gpsimd: wi0h0, wi0h1        then wi1
            #   scalar: consts, xc0h1, xc2, xc3h1   then wo1 (deferred)
            scales_sb = const.tile([128, TOPK], F32)
            nc.scalar.dma_start(out=scales_sb[:], in_=scales[:])
            bias1_sb = const.tile([128, TOPK * HB], F32)
            nc.scalar.dma_start(out=bias1_sb[:], in_=bias1[:])
            if with_bo:
                bo_sb = const.tile([1, D_OUT], BF16)
                nc.scalar.dma_start(out=bo_sb[:], in_=bo_g[:])
                ones_sb = const.tile([1, CH], BF16)
                nc.vector.memset(ones_sb[:], 1.0)
            # Startup DMA layout (empirically best):
            #   gpsimd: wi0h0, wi0h1, wi1
            #   sync:   xc0h0, xc1, xc3h0, wo0 (then the out stores)
            #   scalar: consts, xc0h1, xc2, xc3h1, (wo1 deferred)
            def _xdma(eng, tile_, lo, hi):
                eng.dma_start(out=tile_[:], in_=xt_in[:, lo * CH:hi * CH])

            # wi0 as four sequential quarters on gpsimd: the first quarter
            # lands ~3us before a half would, and the interleaved pass A
            # (db 0-3 twice, then 4-7 twice) gives the later quarters
            # deadlines of t_start+8.3us/10.4us, which they comfortably
            # make even at the 4KB-row service rate.
            wi0_quarters = []
            for q in range(4):
                wq = const.tile([128, 2 * D_HID], BF16, tag=f"wi0q{q}",
                                name=f"wi0q{q}")
                nc.gpsimd.dma_start(
                    out=wq[:],
                    in_=wi[0, :, q * 2 * D_HID:(q + 1) * 2 * D_HID],
                )
                wi0_quarters.append(wq)
            xc0_halves = []
            xc3_halves = []
            for half in range(2):
                xh = xtp.tile([128, KB // 2, CH], BF16, tag=f"xc0h{half}",
                              name=f"xc0h{half}")
                _xdma(nc.sync if half == 0 else nc.scalar,
                      xh, half * 4, (half + 1) * 4)
                xc0_halves.append(xh)
            xt_chunks = [None]
            for c in (1, 2):
                xc = xtp.tile([128, KB, CH], BF16, tag=f"xc{c}", name=f"xc{c}")
                _xdma(nc.sync if c == 1 else nc.scalar,
                      xc, c * KB, (c + 1) * KB)
                xt_chunks.append(xc)
            for half in range(2):
                xh = xtp.tile([128, KB // 2, CH], BF16, tag=f"xc3h{half}",
                              name=f"xc3h{half}")
                _xdma(nc.sync if half == 0 else nc.scalar,
                      xh, 3 * KB + half * 4, 3 * KB + (half + 1) * 4)
                xc3_halves.append(xh)
            wi1_sb = const.tile([128, KB * D_HID], BF16)
            nc.gpsimd.dma_start(out=wi1_sb[:], in_=wi[1])
            wo0_sb = const.tile([128, HB * D_OUT], BF16)
            nc.sync.dma_start(out=wo0_sb[:], in_=wo[0])
            # wo1 allocated now, its load ISSUED from the scalar engine
            # after the pass-A relus (so its issue can't block the scalar
            # sequencer while x loads are still queued)
            wo1_sb = const.tile([128, HB * D_OUT], BF16)
            wo_sb = [wo0_sb, wo1_sb]

            # PE warmup while DMAs stage (HAM -> 8/8 before real matmuls);
            # offsets varied so LDW dedupe keeps each load.
            wz = const.tile([128, 512], BF16, tag="warm")
            nc.vector.memset(wz[:], 0.0)
            pw = ps.tile([128, 512], F32, space="PSUM", tag="ps", bufs=8,
                         name="warm")
            NW = 44
            for i in range(NW):
                o = (i % 4) * 128
                nc.tensor.matmul(pw[:, :128], lhsT=wz[:, o:o + 128],
                                 rhs=wz[:, :128],
                                 start=(i == 0), stop=(i == NW - 1))

            def wi_lhsT(e, db, h):
                if e == 0:
                    return wi0_quarters[db // 2][
                        :, (db % 2) * D_HID + h * 128:
                        (db % 2) * D_HID + (h + 1) * 128]
                return wi1_sb[:, db * D_HID + h * 128:
                              db * D_HID + (h + 1) * 128]

            def xt_rhs(db, c):
                if c == 0:
                    return xc0_halves[db // 4][:, db % 4, :]
                if c == 3:
                    return xc3_halves[db // 4][:, db % 4, :]
                return xt_chunks[c][:, db, :]

            ht_tiles = {}

            def ht_of(c):
                if c not in ht_tiles:
                    ht_tiles[c] = htp.tile([128, TOPK * HB, CH], BF16,
                                           tag="ht", name=f"ht{c}")
                return ht_tiles[c]

            def relu_out(c, e, h, ph):
                nc.scalar.activation(
                    ht_of(c)[:, e * HB + h, :], ph[:],
                    mybir.ActivationFunctionType.Relu,
                    bias=bias1_sb[:, e * HB + h:e * HB + h + 1],
                    scale=scales_sb[:, e:e + 1],
                )

            # --- FFN1 pass A: (e0, c0) with all 8 h-tiles' accumulation
            # groups interleaved: both h-groups consume db 0-3 (first wi0
            # half + first xc0 half) before either touches db 4-7, pushing
            # the second halves' DMA deadline ~4us later.  The kernel
            # start is DMA-paced, so the PE chases arrivals here.
            phsA = [
                ps.tile([128, CH], F32, space="PSUM", tag="ps",
                        name=f"phA_{j}", bufs=8)
                for j in range(4)
            ]
            phsB = [
                ps.tile([128, CH], F32, space="PSUM", tag="ps",
                        name=f"phB_{j}", bufs=8)
                for j in range(4)
            ]
            for half, phs, hbase in ((0, phsA, 0), (0, phsB, 4),
                                     (1, phsA, 0), (1, phsB, 4)):
                for db in range(half * 4, half * 4 + 4):
                    for j in range(4):
                        nc.tensor.matmul(
                            phs[j][:], lhsT=wi_lhsT(0, db, hbase + j),
                            rhs=xt_rhs(db, 0),
                            start=(db == 0), stop=(db == KB - 1),
                        )
                if half == 1:
                    for j in range(4):
                        relu_out(0, 0, hbase + j, phs[j])
            nc.scalar.dma_start(out=wo1_sb[:], in_=wo[1])

            # --- FFN1 main: h-outer, db inner, chunk-group innermost so
            # each wi tile loads once and streams the whole group (LDW
            # dedupe).  All psum tiles share one 8-deep bank rotation, so
            # the groups stay double-buffered against the relu drain.
            def ffn1_h(e, h, chunks):
                phs = {
                    c: ps.tile([128, CH], F32, space="PSUM", tag="ps",
                               name=f"ph{e}_{h}_{c}", bufs=8)
                    for c in chunks
                }
                for db in range(KB):
                    for c in chunks:
                        nc.tensor.matmul(
                            phs[c][:], lhsT=wi_lhsT(e, db, h),
                            rhs=xt_rhs(db, c),
                            start=(db == 0), stop=(db == KB - 1),
                        )
                for c in chunks:
                    relu_out(c, e, h, phs[c])

            for h in range(HB):
                ffn1_h(0, h, range(1, NCH))
            for h in range(HB):
                ffn1_h(1, h, range(NCH))

            # --- FFN2 transposed: out^T[do,tok] = sum_{e,h} Wo^T @ hT
            # (+ bo_g ⊗ ones).  Wo tile stationary, all four chunks
            # streamed per load; psum [do=128, tok=512] per (do, chunk).
            n_mm = TOPK * HB
            for n in range(NB):
                ob = obp.tile([128, TC], BF16, tag="ob", name=f"ob{n}")
                pos = {
                    c: ps.tile([128, CH], F32, space="PSUM", tag="ps",
                               name=f"po{n}_{c}", bufs=8)
                    for c in range(NCH)
                }
                k = 0
                for e in range(TOPK):
                    for h in range(HB):
                        k += 1
                        for c in range(NCH):
                            nc.tensor.matmul(
                                pos[c][:],
                                lhsT=wo_sb[e][:, h * D_OUT + n * 128:
                                              h * D_OUT + (n + 1) * 128],
                                rhs=ht_tiles[c][:, e * HB + h, :],
                                start=(k == 1),
                                stop=(not with_bo and k == n_mm),
                            )
                if with_bo:
                    for c in range(NCH):
                        nc.tensor.matmul(
                            pos[c][:],
                            lhsT=bo_sb[:, n * 128:(n + 1) * 128],
                            rhs=ones_sb[:], start=False, stop=True,
                        )
                # psum->sbuf bf16 copies split across the (otherwise idle)
                # vector and scalar engines so they drain in ~half the time
                for c in range(NCH):
                    dst = ob[:, c * CH:(c + 1) * CH]
                    if c % 2 == 0:
                        nc.vector.tensor_copy(dst, pos[c][:])
                    else:
                        nc.scalar.activation(
                            dst, pos[c][:],
                            mybir.ActivationFunctionType.Copy,
                        )
                if n < NB - 1:
                    for lo in (0, 2 * CH):
                        nc.sync.dma_start(
                            out=out[n * 128:(n + 1) * 128, lo:lo + 2 * CH],
                            in_=ob[:, lo:lo + 2 * CH],
                        )
                else:
                    # last block: store per chunk on alternating queues so
                    # the final stores (on the critical tail) issue in
                    # parallel and each covers only 128KB
                    for c in range(NCH):
                        eng = nc.sync if c % 2 == 0 else nc.gpsimd
                        eng.dma_start(
                            out=out[n * 128:(n + 1) * 128,
                                    c * CH:(c + 1) * CH],
                            in_=ob[:, c * CH:(c + 1) * CH],
                        )

    _dedupe_ldweights(nc)
    _split_multi_waits(nc)
    return nc


_CACHE = {}


def _phase(name, *args):
    key = (name, *args)
    if key not in _CACHE:
        _CACHE[key] = _build_phase1() if name == "p1" else _build_phase2(*args)
    return _CACHE[key]


def _bf16(a):
    return np.asarray(a, np.float32).astype(ml_dtypes.bfloat16)


def _chunk_major(shard):
    """[TC, D_IN] -> [128, NCH*KB*CH] with row p = [c][db][t], so every
    chunk DMA is a single contiguous segment per partition."""
    return np.ascontiguousarray(
        shard.reshape(NCH, CH, KB, 128).transpose(3, 0, 2, 1)
        .reshape(128, NCH * KB * CH)
    )


def kernel(x, noise, gate_w, gate_noise_w, Wi, bi, Wo, bo, _timing=None):
    x = np.asarray(x, np.float32)
    noise = np.asarray(noise, np.float32)
    gate_w = np.asarray(gate_w, np.float32)
    gate_noise_w = np.asarray(gate_noise_w, np.float32)
    bi = np.asarray(bi, np.float32)
    bo = np.asarray(bo, np.float32)

    xb = _bf16(x.reshape(T, D_IN))
    xt_shards = [_chunk_major(xb[c * TC:(c + 1) * TC]) for c in range(N_CORES)]
    core_ids = list(range(N_CORES))

    # ---- phase 1: gate partials (fp8 halves the gate-phase DMA)
    xf8 = x.reshape(T, D_IN).astype(ml_dtypes.float8_e4m3)
    xt8_shards = [
        _chunk_major(xf8[c * TC:(c + 1) * TC]) for c in range(N_CORES)
    ]
    gw_cat = np.concatenate([gate_w, gate_noise_w], axis=1).astype(
        ml_dtypes.float8_e4m3
    )
    # [p, db-pair, 2, 128] layout for DoubleRow (one contiguous row per
    # partition)
    gw_host = np.ascontiguousarray(
        gw_cat.reshape(KB // 2, 2, 128, 128).transpose(2, 0, 1, 3)
        .reshape(128, KB * 128)
    )
    noise_col = noise.reshape(E, 1)
    ident = np.eye(E, dtype=np.float32)
    in1 = [
        {"xt": xt8_shards[c], "gw": gw_host, "noise": noise_col,
         "ident": ident}
        for c in range(N_CORES)
    ]
    r1 = run_bass_kernel_spmd(_phase("p1"), in1, core_ids,
                              **(_timing or {}).get("p1", {}))
    mean_logits = (
        sum(r1.results[c]["part"][0, :].astype(np.float64)
            for c in range(N_CORES)) / T
    ).astype(np.float32)

    # ---- host routing: top-2 + softmax (stable => jax.lax.top_k ties)
    idx = np.argsort(-mean_logits, kind="stable")[:TOPK]
    tv = mean_logits[idx]
    ex = np.exp(tv - tv.max())
    gates = (ex / ex.sum()).astype(np.float32)

    # ---- phase 2: FFN on the two selected experts
    # [e, p, db*D + col] layout: one contiguous row per partition
    wi_sel = np.ascontiguousarray(
        _bf16(np.asarray(Wi)[idx]).reshape(TOPK, KB, 128, D_HID)
        .transpose(0, 2, 1, 3).reshape(TOPK, 128, KB * D_HID)
    )
    wo_sel = np.ascontiguousarray(
        _bf16(np.asarray(Wo)[idx]).reshape(TOPK, HB, 128, D_OUT)
        .transpose(0, 2, 1, 3).reshape(TOPK, 128, HB * D_OUT)
    )
    scales = np.broadcast_to(gates, (128, TOPK)).copy()
    # bias1[p, e*HB+h] = g_e * bi[e_sel, h*128+p]
    bias1 = (gates[:, None] * bi[idx]).reshape(TOPK, HB, 128)
    bias1 = np.ascontiguousarray(bias1.transpose(2, 0, 1).reshape(128, TOPK * HB))
    with_bo = bool(np.any(bo[idx]))
    in2 = [
        {
            "xt": xt_shards[c], "wi": wi_sel, "wo": wo_sel,
            "scales": scales, "bias1": bias1,
        }
        for c in range(N_CORES)
    ]
    if with_bo:
        bo_g = _bf16((gates[:, None] * bo[idx]).sum(0).reshape(1, D_OUT))
        for m in in2:
            m["bo_g"] = bo_g
    r2 = run_bass_kernel_spmd(_phase("p2", with_bo), in2, core_ids,
                              **(_timing or {}).get("p2", {}))
    out = np.concatenate(
        [np.asarray(r2.results[c]["out"]).astype(np.float32).T
         for c in range(N_CORES)], axis=0
    )

    if isinstance(_timing, dict):
        _timing["exec_ns"] = [r1.exec_time_ns, r2.exec_time_ns]
    return out.reshape(B, L, D_OUT).astype(np.float32, copy=False)


# revision 57
# speedup vs baseline: 1.0343x; 1.0221x over previous
"""MoE feed-forward (noisy top-2 gating over 64 experts) on 8 TRN2 NeuronCores.

Strategy (two device phases, host does only the 64-way top-2 bookkeeping):
  Phase 1 (device): tokens sharded 2048/core. Each core computes its shard's
    gate logits  x @ [gate_w | gate_noise_w]  in fp8 (f32 PSUM accumulate),
    applies softplus/noise, reduces over its tokens -> [64] partial sums,
    PE-transposes to [1,64] and stores with a single DMA descriptor.
  Host: sums the 8 partials -> mean logits, top-2 + softmax (matches
    jax.lax.top_k tie semantics via stable argsort), slices + bf16-casts the
    two selected experts' weight tables.
  Phase 2 (device): per core, hT = relu(g_e * (x @ Wi_e)) for both experts
    (gates folded into the relu scale), then out^T[do,tok] = sum_e Wo_e^T hT_e
    with Wo stationary in the PE (4 token-chunks streamed per weight load).
    Output is stored transposed in bf16; the host un-transposes for free.

Perf notes vs the previous version:
  - x is staged host-side in chunk-major contiguous layout so every chunk
    DMA is 128 x 4-8KB descriptors (line rate) instead of 512B scatter.
  - FFN1/FFN2 loops are ordered so consecutive matmuls share the stationary
    operand; a post-BIR pass drops the redundant LDWEIGHTS (each costs
    ~46ns of PE issue time; >700 of them in the old schedule).
  - Phase-1's [64,1] result is PE-transposed to [1,64] so the final store is
    one descriptor instead of 16 (the scattered store waited ~5us).

All matmuls run with fp32 PSUM accumulation (measured end-to-end rel err
~3e-3 vs the fp32 reference; top-2 selection margin is ~4000x the bf16 gate
error on the reference input distribution).
"""

import sys

for _p in ("/opt/trn_rl_repo", "/root/.axon_site/_ro/trn_rl_repo"):
    if _p not in sys.path:
        sys.path.insert(0, _p)

import ml_dtypes
import numpy as np

import concourse.bass as bass
import concourse.mybir as mybir
import concourse.tile as tile
from concourse import bass_utils
from concourse.bass_utils import run_bass_kernel_spmd


def _patch_walrus_args():
    """Allow injecting extra walrus_driver flags via EXTRA_WALRUS_ARGS
    (comma-separated).  Flags given here replace an existing flag with the
    same --name if present."""
    import os

    orig = bass_utils.bir_verify_and_optimise
    if getattr(bass_utils, "_walrus_patched", False):
        return

    def patched(tmpdir, inp="bir.json", outp="file.neff", arch=None, *,
                dve_root=None):
        extra = os.environ.get("EXTRA_WALRUS_ARGS", "")
        if not extra:
            return orig(tmpdir, inp=inp, outp=outp, arch=arch,
                        dve_root=dve_root)
        import concourse.bass_utils as bu

        run_command_orig = bu.run_command

        def run_command_patched(cmd, **kw):
            if cmd and str(cmd[0]).endswith("walrus_driver"):
                new = list(cmd)
                for flag in extra.split(","):
                    if not flag:
                        continue
                    name = flag.split("=")[0]
                    new = [a for a in new if not str(a).startswith(name)]
                    new.append(flag)
                cmd = new
            return run_command_orig(cmd, **kw)

        bu.run_command = run_command_patched
        try:
            return orig(tmpdir, inp=inp, outp=outp, arch=arch,
                        dve_root=dve_root)
        finally:
            bu.run_command = run_command_orig

    bass_utils.bir_verify_and_optimise = patched
    bass_utils._walrus_patched = True


_patch_walrus_args()


def _ensure_ntff_hook():
    """Make trace=True / BASS_TRACE profiling work even when the image's
    antenv package lacks axon_hooks (boot then skips hook registration)."""
    try:
        import antenv.axon_hooks  # noqa: F401
        return
    except ImportError:
        pass
    try:
        import types

        import antenv

        mod = types.ModuleType("antenv.axon_hooks")
        mod._hook = None

        def set_axon_ntff_profile_hook(hook):
            mod._hook = hook

        def get_axon_ntff_profile_hook():
            return mod._hook

        mod.set_axon_ntff_profile_hook = set_axon_ntff_profile_hook
        mod.get_axon_ntff_profile_hook = get_axon_ntff_profile_hook
        sys.modules["antenv.axon_hooks"] = mod
        antenv.axon_hooks = mod
        from trn_agent_boot.trn_boot import _ntff_profile_via_ctypes

        mod._hook = _ntff_profile_via_ctypes("/opt/axon/libaxon_pjrt.so")
    except Exception:
        pass  # profiling degrades gracefully; execution is unaffected


_ensure_ntff_hook()

# ---------------------------------------------------------------- shapes
B, L, D_IN, D_HID, D_OUT = 4, 4096, 1024, 1024, 1024
E, TOPK = 64, 2
N_CORES = 8
T = B * L            # 16384 tokens
TC = T // N_CORES    # 2048 tokens per core
CH = 512             # token chunk (matmul moving free dim = one PSUM bank)
NCH = TC // CH       # 4 chunks per core
KB = D_IN // 128     # 8 contraction blocks
HB = D_HID // 128    # 8 hidden blocks
NB = D_OUT // 128    # 8 output blocks

F32 = mybir.dt.float32
BF16 = mybir.dt.bfloat16
FP8 = mybir.dt.float8e4  # ml_dtypes.float8_e4m3

# ------------------------------------------------- walrus workaround
# The walrus build in this container supports only ONE sync-wait command
# per instruction; Tile attaches multi-wait lists.  Split them: the tail
# drain via a patched _drain_and_barrier, everything else via a BIR
# post-pass inserting single-wait NoOps ahead of multi-wait instructions.
_TILE_PATCHED = False


def _patch_tile_drain():
    global _TILE_PATCHED
    if _TILE_PATCHED:
        return
    _TILE_PATCHED = True

    def _drain_and_barrier(self, tick_clock, wait_clock):
        n1 = self.nc.sync.nop(nofuse=True)
        wait_clock.add_sem_waits(
            n1.ins, tile.ScopedClock({None: tick_clock.global_clock})
        )
        waits = list(n1.ins.sync_info.on_wait) if n1.ins.sync_info else []
        if len(waits) > 1:
            n1.ins.sync_info.on_wait = waits[:1]
            for i in range(1, len(waits)):
                nx = self.nc.sync.nop(nofuse=True)
                nx.ins.sync_info = mybir.SyncInfo(on_wait=[waits[i]], on_update=[])
        self.nc.sync.drain()
        self.nc.all_engine_barrier()
        assert self.sems is not None
        popped = self.nc._tile_sem_poison_stack.pop()
        assert popped is self._sem_poison
        # python-side bookkeeping only — the device-side clear
        # (gpsimd dma_reset + sem RANGE_CLEAR, ~2-3us of q7-launch
        # latency) and the trailing barrier are redundant with the
        # walrus postamble's full semaphore wipe
        sems = [s.num if hasattr(s, "num") else s
                for s in self.sems.allocated().values()]
        self.nc._state.prepend_free_semaphores(sems)
        for poison_set in self.nc._tile_sem_poison_stack:
            poison_set.update(sems)

    tile.TileContext._drain_and_barrier = _drain_and_barrier


def _split_multi_waits(nc):
    n_split = 0
    for f in nc.m.functions:
        for bb in f.blocks:
            insts = list(bb.instructions)
            out = []
            for inst in insts:
                si = inst.sync_info
                if si is not None and si.on_wait and len(si.on_wait) > 1:
                    waits = list(si.on_wait)
                    for w in waits[:-1]:
                        nop = mybir.InstNoOp(
                            name=f"{inst.name}-ws{n_split}", ins=[], outs=[]
                        )
                        nop.engine = inst.engine
                        nop.sync_info = mybir.SyncInfo(on_wait=[w], on_update=[])
                        out.append(nop)
                        n_split += 1
                    si.on_wait = waits[-1:]
                out.append(inst)
            if len(out) != len(insts):
                bb.instructions[:] = out
    return n_split


def _dedupe_ldweights(nc):
    """Drop InstLdweights that reload the exact weights already resident in
    the PE array (same AP/mode as the immediately preceding load, with no
    intervening PE-array-clobbering op).  The PE keeps its stationary
    operand across matmuls, so consecutive matmuls sharing lhsT only need
    the first load; each removed LDWEIGHTS saves ~46ns of PE issue time.
    Sync waits/updates on a removed load are transferred to the following
    instruction (the matmul), then _split_multi_waits handles overflow."""
    n_removed = 0
    for f in nc.m.functions:
        for bb in f.blocks:
            insts = list(bb.instructions)
            out = []
            prev_sig = None
            carry = None  # sync_info of removed LDW awaiting a new home
            for inst in insts:
                is_pe = getattr(inst, "engine", None) == mybir.EngineType.PE
                if isinstance(inst, mybir.InstLdweights):
                    sig = (
                        str(inst.ins[0]),
                        str(inst.perf_mode),
                        bool(inst.is_transpose),
                    )
                    if inst.is_transpose:
                        prev_sig = None
                        out.append(inst)
                        continue
                    if sig == prev_sig:
                        si = inst.sync_info
                        if si is not None and (si.on_wait or si.on_update):
                            if carry is None:
                                carry = mybir.SyncInfo(on_wait=[], on_update=[])
                            carry.on_wait.extend(si.on_wait)
                            carry.on_update.extend(si.on_update)
                        n_removed += 1
                        continue
                    prev_sig = sig
                    out.append(inst)
                    continue
                if is_pe:
                    if isinstance(inst, mybir.InstMatmult):
                        if inst.is_transpose:
                            prev_sig = None
                    elif not isinstance(
                        inst, (mybir.InstEventSemaphore, mybir.InstNoOp)
                    ):
                        # drains/branches/etc: don't assume array state
                        prev_sig = None
                    if carry is not None:
                        si = inst.sync_info
                        if si is None:
                            inst.sync_info = carry
                        else:
                            si.on_wait.extend(carry.on_wait)
                            si.on_update.extend(carry.on_update)
                        carry = None
                out.append(inst)
            assert carry is None, "removed LDW waits had no following PE inst"
            if len(out) != len(insts):
                bb.instructions[:] = out
    return n_removed


# ------------------------------------------------------------ builders
def _build_phase1():
    """Gate partials: per core [1,64] f32 = sum over its 2048 tokens of
    x@gate_w + softplus(x@gate_noise_w)*noise   (fp8 matmul, f32 psum).

    fp8-e4m3 is safe here: quantization noise averages over 16384 tokens
    (measured mean-logit err 1.3e-3 vs 0.216 top-2/3 margin, and 8e-5
    absolute error on the softmax gates)."""
    _patch_tile_drain()
    nc = bass.Bass("TRN2", target_bir_lowering=False, debug=False,
                   num_devices=N_CORES)
    # host layout: row p = [c][db][t] so each chunk DMA is one contiguous
    # 4KB segment per partition (line rate; the old (db p) t rearrange
    # produced 512B descriptors and ran at ~100GB/s)
    xt_in = nc.dram_tensor("xt", [128, NCH * KB * CH], FP8,
                           kind="ExternalInput")
    gw = nc.dram_tensor("gw", [128, KB * 128], FP8, kind="ExternalInput")
    noise = nc.dram_tensor("noise", [E, 1], F32, kind="ExternalInput")
    ident = nc.dram_tensor("ident", [E, E], F32, kind="ExternalInput")
    part = nc.dram_tensor("part", [1, E], F32, kind="ExternalOutput")

    with tile.TileContext(nc) as tc:
        with (
            tc.tile_pool(name="const", bufs=1) as const,
            tc.tile_pool(name="xt", bufs=1) as xtp,
            tc.tile_pool(name="ps", bufs=1, space="PSUM") as psp,
            tc.tile_pool(name="sb", bufs=3) as sbp,
            tc.tile_pool(name="red", bufs=NCH + 2) as redp,
        ):
            # gw staged as [p, db-pair, 2, 128] for DoubleRow matmuls
            # (fp8 high-perf mode: 2 contraction k-tiles per instruction)
            gw_sb = const.tile([128, KB // 2, 2, 128], FP8)
            nc.scalar.dma_start(out=gw_sb[:], in_=gw[:])
            noise_sb = const.tile([E, 1], F32)
            nc.scalar.dma_start(out=noise_sb[:], in_=noise[:])
            ident_sb = const.tile([E, E], F32)
            nc.scalar.dma_start(out=ident_sb[:], in_=ident[:])

            # load chunk PAIRS (8KB rows — 4KB fp8 rows halve the queue
            # service rate) on the two fast FIFOs, consts on scalar
            pair_tiles = []
            for p_ in range(2):
                pt_ = xtp.tile([128, 2 * KB, CH], FP8, tag=f"xp{p_}",
                               name=f"xp{p_}")
                (nc.sync if p_ == 0 else nc.gpsimd).dma_start(
                    out=pt_[:],
                    in_=xt_in[:, p_ * 2 * KB * CH:(p_ + 1) * 2 * KB * CH],
                )
                pair_tiles.append(pt_)

            def xt_rhs(c, db):
                return pair_tiles[c // 2][:, (c % 2) * KB + db, :]

            # PE warmup while DMAs stage (HAM clock gate -> 8/8), long
            # enough to bridge to the first chunk's arrival so the PE
            # doesn't idle (idling drops the clock back to mid-pstate);
            # offsets varied so the LDW-dedupe pass keeps each load.
            wz = const.tile([128, 512], BF16, tag="warm")
            nc.vector.memset(wz[:], 0.0)
            pw = psp.tile([128, 512], F32, space="PSUM", tag="warm")
            NW = 40
            for i in range(NW):
                o = (i % 4) * 128
                nc.tensor.matmul(pw[:, :128], lhsT=wz[:, o:o + 128],
                                 rhs=wz[:, :128],
                                 start=(i == 0), stop=(i == NW - 1))

            partials = []
            for c in range(NCH):
                ps_g = psp.tile([128, CH], F32, space="PSUM", tag="g",
                                name=f"ps_g{c}", bufs=3)
                for db2 in range(KB // 2):
                    base = (c % 2) * KB + 2 * db2
                    nc.tensor.matmul(
                        ps_g[:], lhsT=gw_sb[:, db2, :, :],
                        rhs=pair_tiles[c // 2][:, base:base + 2, :],
                        start=(db2 == 0), stop=(db2 == KB // 2 - 1),
                        perf_mode=mybir.MatmulPerfMode.DoubleRow,
                    )
                # softplus(v) = ln(exp(v) + 1) — this walrus's ACT tables
                # have no native softplus; exp/ln share one func set.
                # Gate pre-activations are O(10), so exp cannot overflow.
                ex = sbp.tile([E, CH], F32)
                nc.scalar.activation(
                    ex[:], ps_g[E:2 * E, :], mybir.ActivationFunctionType.Exp,
                )
                sp = sbp.tile([E, CH], F32)
                nc.scalar.activation(
                    sp[:], ex[:], mybir.ActivationFunctionType.Ln, bias=1.0,
                )
                comb = sbp.tile([E, CH], F32)
                pc = redp.tile([E, 1], F32, tag="partial")
                nc.vector.scalar_tensor_tensor(
                    out=comb[:], in0=sp[:], scalar=noise_sb[:, :1],
                    in1=ps_g[:E, :],
                    op0=mybir.AluOpType.mult, op1=mybir.AluOpType.add,
                    accum_out=pc[:],
                )
                partials.append(pc)
            while len(partials) > 1:
                nxt = []
                for i in range(0, len(partials) - 1, 2):
                    s = redp.tile([E, 1], F32, tag="sum")
                    nc.vector.tensor_add(s[:], partials[i][:], partials[i + 1][:])
                    nxt.append(s)
                if len(partials) % 2:
                    nxt.append(partials[-1])
                partials = nxt
            # [64,1] -> [1,64] on the PE so the store is one descriptor
            # (the scattered 64-partition store waited ~5us on completion)
            pt = psp.tile([1, E], F32, space="PSUM", tag="pt")
            nc.tensor.transpose(pt[:], partials[0][:], ident_sb[:])
            row = redp.tile([1, E], F32, tag="row")
            nc.vector.tensor_copy(row[:], pt[:])
            nc.sync.dma_start(out=part[:], in_=row[:])

    _dedupe_ldweights(nc)
    _split_multi_waits(nc)
    return nc


def _build_phase2(with_bo):
    """FFN over the two selected experts, token-sharded, gates folded in.

    FFN1: hT[e,h] = relu(g_e*(x @ Wi_e))^T per 128-row h-block, psum [dh,tok].
    FFN2 runs transposed: out^T[do,tok] += Wo[e,h,do]^T @ hT[e,h] with the
    Wo tile stationary, streaming all four 512-token chunks per load; the
    host un-transposes the bf16 result for free.

    Loop order maximizes stationary-operand reuse (LDW dedupe): FFN1 e0/c0
    runs db-outer so the PE consumes wi0 parts the moment they land, the
    rest runs h-outer with db inner and token-chunks innermost.
    """
    _patch_tile_drain()
    nc = bass.Bass("TRN2", target_bir_lowering=False, debug=False,
                   num_devices=N_CORES)
    # host layout: row p = [c][db][t], contiguous 8KB per partition chunk
    xt_in = nc.dram_tensor("xt", [128, NCH * KB * CH], BF16,
                           kind="ExternalInput")
    # host-contiguous layouts: row p holds every block's slice for that
    # partition, so each load is 128 long contiguous descriptors
    wi = nc.dram_tensor("wi", [TOPK, 128, KB * D_HID], BF16,
                        kind="ExternalInput")
    wo = nc.dram_tensor("wo", [TOPK, 128, HB * D_OUT], BF16,
                        kind="ExternalInput")
    scales = nc.dram_tensor("scales", [128, TOPK], F32, kind="ExternalInput")
    bias1 = nc.dram_tensor("bias1", [128, TOPK * HB], F32, kind="ExternalInput")
    if with_bo:
        bo_g = nc.dram_tensor("bo_g", [1, D_OUT], BF16, kind="ExternalInput")
    # transposed output, bf16; host transposes back (free) and upcasts
    out = nc.dram_tensor("out", [D_OUT, TC], BF16, kind="ExternalOutput")

    with tile.TileContext(nc) as tc:
        with (
            tc.tile_pool(name="const", bufs=1) as const,
            tc.tile_pool(name="xt", bufs=1) as xtp,
            tc.tile_pool(name="ps", bufs=1, space="PSUM") as ps,
            tc.tile_pool(name="ht", bufs=NCH) as htp,
            tc.tile_pool(name="ob", bufs=2) as obp,
        ):
            # Per-core DMA is bandwidth-capped and each ACTIVE queue gets
            # an ~equal share, so the startup-critical 6MB (x + wi0) is
            # spread evenly over all three queues in deadline order, and
            # the late-needed loads (wi1, wo) are queued BEHIND them on
            # the same queues.  Queue service rate also scales with
            # descriptor size — keep rows >= 4KB.
            #   sync:   xc0h0, xc1, xc3h0   then wo0
            #   gpsimd: wi0h0, wi0h1        then wi1
            #   scalar: consts, xc0h1, xc2, xc3h1   then wo1 (deferred)
            scales_sb = const.tile([128, TOPK], F32)
            nc.scalar.dma_start(out=scales_sb[:], in_=scales[:])
            bias1_sb = const.tile([128, TOPK * HB], F32)
            nc.scalar.dma_start(out=bias1_sb[:], in_=bias1[:])
            if with_bo:
                bo_sb = const.tile([1, D_OUT], BF16)
                nc.scalar.dma_start(out=bo_sb[:], in_=bo_g[:])
                ones_sb = const.tile([1, CH], BF16)
                nc.vector.memset(ones_sb[:], 1.0)
            # Startup DMA layout (empirically best):
            #   gpsimd: wi0h0, wi0h1, wi1
            #   sync:   xc0h0, xc1, xc3h0, wo0 (then the out stores)
            #   scalar: consts, xc0h1, xc2, xc3h1, (wo1 deferred)
            def _xdma(eng, tile_, lo, hi):
                eng.dma_start(out=tile_[:], in_=xt_in[:, lo * CH:hi * CH])

            # wi0 on gpsimd as [db0-3 half, db45 quarter, db67 quarter]:
            # the first piece uses 8KB rows (fastest service) to open pass
            # A early; the interleaved pass A (db 0-3 twice, then 4-7
            # twice) gives the tail quarters deadlines of t_start+8.3us
            # and +10.4us, which they make at the 4KB-row rate.
            wi0_h0 = const.tile([128, 4 * D_HID], BF16, tag="wi0fh",
                                name="wi0fh")
            nc.gpsimd.dma_start(out=wi0_h0[:], in_=wi[0, :, :4 * D_HID])
            wi0_tailq = []
            for q in (2, 3):
                wq = const.tile([128, 2 * D_HID], BF16, tag=f"wi0q{q}",
                                name=f"wi0q{q}")
                nc.gpsimd.dma_start(
                    out=wq[:],
                    in_=wi[0, :, q * 2 * D_HID:(q + 1) * 2 * D_HID],
                )
                wi0_tailq.append(wq)
            xc0_halves = []
            xc3_halves = []
            for half in range(2):
                xh = xtp.tile([128, KB // 2, CH], BF16, tag=f"xc0h{half}",
                              name=f"xc0h{half}")
                _xdma(nc.sync if half == 0 else nc.scalar,
                      xh, half * 4, (half + 1) * 4)
                xc0_halves.append(xh)
            xt_chunks = [None]
            for c in (1, 2):
                xc = xtp.tile([128, KB, CH], BF16, tag=f"xc{c}", name=f"xc{c}")
                _xdma(nc.sync if c == 1 else nc.scalar,
                      xc, c * KB, (c + 1) * KB)
                xt_chunks.append(xc)
            for half in range(2):
                xh = xtp.tile([128, KB // 2, CH], BF16, tag=f"xc3h{half}",
                              name=f"xc3h{half}")
                _xdma(nc.sync if half == 0 else nc.gpsimd,
                      xh, 3 * KB + half * 4, 3 * KB + (half + 1) * 4)
                xc3_halves.append(xh)
            wi1_sb = const.tile([128, KB * D_HID], BF16)
            nc.gpsimd.dma_start(out=wi1_sb[:], in_=wi[1])
            wo0_sb = const.tile([128, HB * D_OUT], BF16)
            nc.sync.dma_start(out=wo0_sb[:], in_=wo[0])
            # wo1 allocated now, its load ISSUED from the scalar engine
            # after the pass-A relus (so its issue can't block the scalar
            # sequencer while x loads are still queued)
            wo1_sb = const.tile([128, HB * D_OUT], BF16)
            wo_sb = [wo0_sb, wo1_sb]

            # PE warmup while DMAs stage (HAM -> 8/8 before real matmuls);
            # offsets varied so LDW dedupe keeps each load.
            wz = const.tile([128, 512], BF16, tag="warm")
            nc.vector.memset(wz[:], 0.0)
            pw = ps.tile([128, 512], F32, space="PSUM", tag="ps", bufs=8,
                         name="warm")
            NW = 50
            for i in range(NW):
                o = (i % 4) * 128
                nc.tensor.matmul(pw[:, :128], lhsT=wz[:, o:o + 128],
                                 rhs=wz[:, :128],
                                 start=(i == 0), stop=(i == NW - 1))

            def wi_lhsT(e, db, h):
                if e == 0:
                    if db < 4:
                        return wi0_h0[:, db * D_HID + h * 128:
                                      db * D_HID + (h + 1) * 128]
                    return wi0_tailq[(db - 4) // 2][
                        :, (db % 2) * D_HID + h * 128:
                        (db % 2) * D_HID + (h + 1) * 128]
                return wi1_sb[:, db * D_HID + h * 128:
                              db * D_HID + (h + 1) * 128]

            def xt_rhs(db, c):
                if c == 0:
                    return xc0_halves[db // 4][:, db % 4, :]
                if c == 3:
                    return xc3_halves[db // 4][:, db % 4, :]
                return xt_chunks[c][:, db, :]

            ht_tiles = {}

            def ht_of(c):
                if c not in ht_tiles:
                    ht_tiles[c] = htp.tile([128, TOPK * HB, CH], BF16,
                                           tag="ht", name=f"ht{c}")
                return ht_tiles[c]

            def relu_out(c, e, h, ph):
                nc.scalar.activation(
                    ht_of(c)[:, e * HB + h, :], ph[:],
                    mybir.ActivationFunctionType.Relu,
                    bias=bias1_sb[:, e * HB + h:e * HB + h + 1],
                    scale=scales_sb[:, e:e + 1],
                )

            # --- FFN1 pass A: (e0, c0) with all 8 h-tiles' accumulation
            # groups interleaved: both h-groups consume db 0-3 (first wi0
            # half + first xc0 half) before either touches db 4-7, pushing
            # the second halves' DMA deadline ~4us later.  The kernel
            # start is DMA-paced, so the PE chases arrivals here.
            phsA = [
                ps.tile([128, CH], F32, space="PSUM", tag="ps",
                        name=f"phA_{j}", bufs=8)
                for j in range(4)
            ]
            phsB = [
                ps.tile([128, CH], F32, space="PSUM", tag="ps",
                        name=f"phB_{j}", bufs=8)
                for j in range(4)
            ]
            for half, phs, hbase in ((0, phsA, 0), (0, phsB, 4),
                                     (1, phsA, 0), (1, phsB, 4)):
                for db in range(half * 4, half * 4 + 4):
                    for j in range(4):
                        nc.tensor.matmul(
                            phs[j][:], lhsT=wi_lhsT(0, db, hbase + j),
                            rhs=xt_rhs(db, 0),
                            start=(db == 0), stop=(db == KB - 1),
                        )
                if half == 1:
                    for j in range(4):
                        relu_out(0, 0, hbase + j, phs[j])
            nc.scalar.dma_start(out=wo1_sb[:], in_=wo[1])

            # --- FFN1 main: h-outer, db inner, chunk-group innermost so
            # each wi tile loads once and streams the whole group (LDW
            # dedupe).  All psum tiles share one 8-deep bank rotation, so
            # the groups stay double-buffered against the relu drain.
            def ffn1_h(e, h, chunks):
                phs = {
                    c: ps.tile([128, CH], F32, space="PSUM", tag="ps",
                               name=f"ph{e}_{h}_{c}", bufs=8)
                    for c in chunks
                }
                for db in range(KB):
                    for c in chunks:
                        nc.tensor.matmul(
                            phs[c][:], lhsT=wi_lhsT(e, db, h),
                            rhs=xt_rhs(db, c),
                            start=(db == 0), stop=(db == KB - 1),
                        )
                for c in chunks:
                    relu_out(c, e, h, phs[c])

            for h in range(HB):
                ffn1_h(0, h, range(1, NCH))
            for h in range(HB):
                ffn1_h(1, h, range(NCH))

            # --- FFN2 transposed: out^T[do,tok] = sum_{e,h} Wo^T @ hT
            # (+ bo_g ⊗ ones).  Wo tile stationary, all four chunks
            # streamed per load; psum [do=128, tok=512] per (do, chunk).
            n_mm = TOPK * HB
            for n in range(NB):
                ob = obp.tile([128, TC], BF16, tag="ob", name=f"ob{n}")
                pos = {
                    c: ps.tile([128, CH], F32, space="PSUM", tag="ps",
                               name=f"po{n}_{c}", bufs=8)
                    for c in range(NCH)
                }
                k = 0
                for e in range(TOPK):
                    for h in range(HB):
                        k += 1
                        for c in range(NCH):
                            nc.tensor.matmul(
                                pos[c][:],
                                lhsT=wo_sb[e][:, h * D_OUT + n * 128:
                                              h * D_OUT + (n + 1) * 128],
                                rhs=ht_tiles[c][:, e * HB + h, :],
                                start=(k == 1),
                                stop=(not with_bo and k == n_mm),
                            )
                if with_bo:
                    for c in range(NCH):
                        nc.tensor.matmul(
                            pos[c][:],
                            lhsT=bo_sb[:, n * 128:(n + 1) * 128],
                            rhs=ones_sb[:], start=False, stop=True,
                        )
                # psum->sbuf bf16 copies split across the (otherwise idle)
                # vector and scalar engines so they drain in ~half the time
                for c in range(NCH):
                    dst = ob[:, c * CH:(c + 1) * CH]
                    if c % 2 == 0:
                        nc.vector.tensor_copy(dst, pos[c][:])
                    else:
                        nc.scalar.activation(
                            dst, pos[c][:],
                            mybir.ActivationFunctionType.Copy,
                        )
                if n < NB - 1:
                    for lo in (0, 2 * CH):
                        nc.sync.dma_start(
                            out=out[n * 128:(n + 1) * 128, lo:lo + 2 * CH],
                            in_=ob[:, lo:lo + 2 * CH],
                        )
                else:
                    # last block: store per chunk on alternating queues so
                    # the final stores (on the critical tail) issue in
                    # parallel and each covers only 128KB
                    for c in range(NCH):
                        eng = nc.sync if c % 2 == 0 else nc.gpsimd
                        eng.dma_start(
                            out=out[n * 128:(n + 1) * 128,
                                    c * CH:(c + 1) * CH],
                            in_=ob[:, c * CH:(c + 1) * CH],
                        )

    _dedupe_ldweights(nc)
    _split_multi_waits(nc)
    return nc


_CACHE = {}


def _phase(name, *args):
    key = (name, *args)
    if key not in _CACHE:
        _CACHE[key] = _build_phase1() if name == "p1" else _build_phase2(*args)
    return _CACHE[key]


def _bf16(a):
    return np.asarray(a, np.float32).astype(ml_dtypes.bfloat16)


def _chunk_major(shard):
    """[TC, D_IN] -> [128, NCH*KB*CH] with row p = [c][db][t], so every
    chunk DMA is a single contiguous segment per partition."""
    return np.ascontiguousarray(
        shard.reshape(NCH, CH, KB, 128).transpose(3, 0, 2, 1)
        .reshape(128, NCH * KB * CH)
    )


def kernel(x, noise, gate_w, gate_noise_w, Wi, bi, Wo, bo, _timing=None):
    x = np.asarray(x, np.float32)
    noise = np.asarray(noise, np.float32)
    gate_w = np.asarray(gate_w, np.float32)
    gate_noise_w = np.asarray(gate_noise_w, np.float32)
    bi = np.asarray(bi, np.float32)
    bo = np.asarray(bo, np.float32)

    xb = _bf16(x.reshape(T, D_IN))
    xt_shards = [_chunk_major(xb[c * TC:(c + 1) * TC]) for c in range(N_CORES)]
    core_ids = list(range(N_CORES))

    # ---- phase 1: gate partials (fp8 halves the gate-phase DMA)
    xf8 = x.reshape(T, D_IN).astype(ml_dtypes.float8_e4m3)
    xt8_shards = [
        _chunk_major(xf8[c * TC:(c + 1) * TC]) for c in range(N_CORES)
    ]
    gw_cat = np.concatenate([gate_w, gate_noise_w], axis=1).astype(
        ml_dtypes.float8_e4m3
    )
    # [p, db-pair, 2, 128] layout for DoubleRow (one contiguous row per
    # partition)
    gw_host = np.ascontiguousarray(
        gw_cat.reshape(KB // 2, 2, 128, 128).transpose(2, 0, 1, 3)
        .reshape(128, KB * 128)
    )
    noise_col = noise.reshape(E, 1)
    ident = np.eye(E, dtype=np.float32)
    in1 = [
        {"xt": xt8_shards[c], "gw": gw_host, "noise": noise_col,
         "ident": ident}
        for c in range(N_CORES)
    ]
    r1 = run_bass_kernel_spmd(_phase("p1"), in1, core_ids,
                              **(_timing or {}).get("p1", {}))
    mean_logits = (
        sum(r1.results[c]["part"][0, :].astype(np.float64)
            for c in range(N_CORES)) / T
    ).astype(np.float32)

    # ---- host routing: top-2 + softmax (stable => jax.lax.top_k ties)
    idx = np.argsort(-mean_logits, kind="stable")[:TOPK]
    tv = mean_logits[idx]
    ex = np.exp(tv - tv.max())
    gates = (ex / ex.sum()).astype(np.float32)

    # ---- phase 2: FFN on the two selected experts
    # [e, p, db*D + col] layout: one contiguous row per partition
    wi_sel = np.ascontiguousarray(
        _bf16(np.asarray(Wi)[idx]).reshape(TOPK, KB, 128, D_HID)
        .transpose(0, 2, 1, 3).reshape(TOPK, 128, KB * D_HID)
    )
    wo_sel = np.ascontiguousarray(
        _bf16(np.asarray(Wo)[idx]).reshape(TOPK, HB, 128, D_OUT)
        .transpose(0, 2, 1, 3).reshape(TOPK, 128, HB * D_OUT)
    )
    scales = np.broadcast_to(gates, (128, TOPK)).copy()
    # bias1[p, e*HB+h] = g_e * bi[e_sel, h*128+p]
    bias1 = (gates[:, None] * bi[idx]).reshape(TOPK, HB, 128)
    bias1 = np.ascontiguousarray(bias1.transpose(2, 0, 1).reshape(128, TOPK * HB))
    with_bo = bool(np.any(bo[idx]))
    in2 = [
        {
            "xt": xt_shards[c], "wi": wi_sel, "wo": wo_sel,
            "scales": scales, "bias1": bias1,
        }
        for c in range(N_CORES)
    ]
    if with_bo:
        bo_g = _bf16((gates[:, None] * bo[idx]).sum(0).reshape(1, D_OUT))
        for m in in2:
            m["bo_g"] = bo_g
    r2 = run_bass_kernel_spmd(_phase("p2", with_bo), in2, core_ids,
                              **(_timing or {}).get("p2", {}))
    out = np.concatenate(
        [np.asarray(r2.results[c]["out"]).astype(np.float32).T
         for c in range(N_CORES)], axis=0
    )

    if isinstance(_timing, dict):
        _timing["exec_ns"] = [r1.exec_time_ns, r2.exec_time_ns]
    return out.reshape(B, L, D_OUT).astype(np.float32, copy=False)
